# revision 25
# baseline (speedup 1.0000x reference)
"""BrokenBiasAttention Trainium2 kernel (8-core SPMD).

Sharding: core c -> batch b=c//2, query-row-half r=c%2 (1024 of 2048 rows).
Each core computes q for its rows, k/v for the whole batch, full 8-head
attention for its rows, and the output projection for its rows. Outputs are
disjoint row blocks -> gather is pure concatenation.

Device algorithm (per core):
  - all matmuls in bf16 (weights/x cast on host)
  - scores^T tiles [krow 128, qrow 512] via row-packed K=32 matmuls
  - softmax without max-subtraction (scores bounded ~|10|):
      attn_un = exp(s) * expF,  expF = exp(bias) computed ON HOST, shipped
      bf16, DMA-gathered on device into per-head-pair SBUF tiles.
  - continuous 128-iteration software pipeline over (qc, g2, kt):
      scores(it) -> exp(it) -> mul(it) -> attn@v(it-4); no per-block drain.
  - attn@v + rowsum via column-tiled matmuls accumulating in one PSUM bank
  - normalize: reciprocal_approx_fast straight from PSUM + SBUF->SBUF
    partition-shift DMA broadcast.
"""

import math
import sys

import numpy as np

if "/opt/trn_rl_repo" not in sys.path:
    sys.path.insert(0, "/opt/trn_rl_repo")

N = 2048
C = 256
NH = 8
HD = 32
B = 4
QR = 1024  # q rows per core

_NC = None


def _build_nc(dbg=False):
    import concourse.bass as bass
    import concourse.tile as tile
    from concourse import bacc, mybir
    from concourse.bass import ds, ts

    f32 = mybir.dt.float32
    bf16 = mybir.dt.bfloat16
    EXP = mybir.ActivationFunctionType.Exp

    nc = bacc.Bacc(None, target_bir_lowering=False, debug=False)

    xT = nc.dram_tensor("xT", [C, N], bf16, kind="ExternalInput")
    xTq = nc.dram_tensor("xTq", [C, QR], bf16, kind="ExternalInput")
    Wq_d = nc.dram_tensor("Wq", [C, C], bf16, kind="ExternalInput")
    Wk_d = nc.dram_tensor("Wk", [C, C], bf16, kind="ExternalInput")
    Wv_d = nc.dram_tensor("Wv", [C, C], bf16, kind="ExternalInput")
    Wo_d = nc.dram_tensor("Wo", [C, C], bf16, kind="ExternalInput")
    # host-exp'd bias table: TWE[hp, p, f] = flat exp(TW[2hp:2hp+2]) where
    # TW[h, rdw(11), w2(16), rh(31), w1(16)]
    TWE_d = nc.dram_tensor("TWE", [4, 128, 1364], bf16, kind="ExternalInput")
    out_d = nc.dram_tensor("out", [QR, C], f32, kind="ExternalOutput")

    LAG = 5  # av lags scores by LAG pipeline iterations

    with tile.TileContext(nc) as tc:
        with (
            tc.tile_pool(name="consts", bufs=1) as consts,
            tc.tile_pool(name="expfp", bufs=1) as expfp,
            tc.tile_pool(name="xp", bufs=6) as xp,
            tc.tile_pool(name="kqv", bufs=1) as kqv,
            tc.tile_pool(name="ep", bufs=8) as ep,
            tc.tile_pool(name="rp", bufs=2) as rp,
            tc.tile_pool(name="otp", bufs=1) as otp,
            tc.tile_pool(name="stp", bufs=2) as stp,
            tc.tile_pool(name="spsum", bufs=3, space="PSUM") as spsum,
            tc.tile_pool(name="apsum", bufs=2, space="PSUM") as apsum,
        ):
            # ---- constants + weights first (small DMAs) ----
            w_sb = {}
            for name, d in (("Wq", Wq_d), ("Wk", Wk_d), ("Wv", Wv_d), ("Wo", Wo_d)):
                t = consts.tile([128, 2, C], bf16, tag=f"w_{name}", name=f"w_{name}")
                nc.sync.dma_start(out=t, in_=d[:].rearrange("(ch p) n -> p ch n", p=128))
                w_sb[name] = t
            ones_sb = consts.tile([128, 32], bf16, tag="ones")
            nc.vector.memset(ones_sb, 1.0)

            # ---- expF gather: straight from host-exp'd TWE, per head-pair.
            # One DMA per head-pair (5-D access pattern covering all 8
            # h2p partition blocks), spread across idle engine queues so
            # nothing queues behind a dependent DMA.  hp0-2 are emitted
            # before the projections (scalar/gpsimd queues are idle);
            # hp3 goes on the sync queue after the x loads.
            twe_ap = TWE_d[:]
            expf_tiles = []
            for hp in range(4):
                t = expfp.tile([128, 2 * 11 * 384], bf16, tag=f"expf{hp}",
                               name=f"expf{hp}")
                expf_tiles.append(t)
            gather_eng = [nc.scalar, nc.gpsimd, nc.gpsimd, nc.gpsimd]

            def emit_gather(hp):
                ev_t = expf_tiles[hp].rearrange(
                    "p (h r f) -> p h r f", h=2, r=11, f=384
                )
                for h2p in range(8):
                    gap = bass.AP(
                        tensor=twe_ap.tensor,
                        offset=twe_ap.offset + hp * 174592 + (7 - h2p) * 16,
                        ap=[
                            [496, 16],    # w2 (partition)
                            [7936, 22],   # (h in pair, rdw) merged
                            [1, 384],     # (rh-window, w1) contiguous run
                        ],
                    )
                    gather_eng[hp].dma_start(out=ev_t[ds(16 * h2p, 16)], in_=gap)

            emit_gather(0)

            # ---- projections (all bf16) ----
            kT_sb = [kqv.tile([128, N], bf16, tag=f"kT{m}", name=f"kT{m}")
                     for m in range(2)]
            qT_sb = [kqv.tile([128, QR], bf16, tag=f"qT{m}", name=f"qT{m}")
                     for m in range(2)]
            v_sb = kqv.tile([128, 16, C], bf16, tag="v")
            qscale = 1.0 / math.sqrt(HD)

            # all x loads issued upfront (xp bufs == 6 == tile count)
            xTq_r = xTq[:].rearrange("(ch p) n -> p ch n", p=128)
            xT_r = xT[:].rearrange("(ch p) n -> p ch n", p=128)
            xq_tiles, xc_tiles = [], []
            for j in range(QR // 512):
                xq = xp.tile([128, 2, 512], bf16, tag="x", name=f"xq{j}")
                nc.sync.dma_start(out=xq, in_=xTq_r[:, :, ds(512 * j, 512)])
                xq_tiles.append(xq)
            for j in range(N // 512):
                xc = xp.tile([128, 2, 512], bf16, tag="x", name=f"xc{j}")
                nc.sync.dma_start(out=xc, in_=xT_r[:, :, ds(512 * j, 512)])
                xc_tiles.append(xc)

            # Gate the gpsimd-queue gathers behind the last x load so they
            # don't steal HBM/DGE bandwidth from the projection inputs,
            # then let them rip at full rate (gpsimd queue is otherwise idle).
            gate = consts.tile([16, 16], bf16, tag="gate")
            nc.gpsimd.dma_start(out=gate, in_=xc_tiles[-1][ds(0, 16), 0, ds(0, 16)])
            for hp in range(1, 4):
                emit_gather(hp)

            def emit_qproj(j):
                xq = xq_tiles[j]
                for m in range(2):
                    ps = spsum.tile([128, 1024], f32, tag="s", name=f"qp{j}_{m}")
                    for ch in range(2):
                        nc.tensor.matmul(
                            ps[:, :512],
                            lhsT=w_sb["Wq"][:, ch, ts(m, 128)],
                            rhs=xq[:, ch, :],
                            start=(ch == 0),
                            stop=(ch == 1),
                        )
                    nc.vector.tensor_scalar_mul(
                        qT_sb[m][:, ds(512 * j, 512)], ps[:, :512], qscale
                    )

            def emit_kvproj(j):
                xc = xc_tiles[j]
                for m in range(2):
                    ps = spsum.tile([128, 1024], f32, tag="s", name=f"kp{j}_{m}")
                    for ch in range(2):
                        nc.tensor.matmul(
                            ps[:, :512],
                            lhsT=w_sb["Wk"][:, ch, ts(m, 128)],
                            rhs=xc[:, ch, :],
                            start=(ch == 0),
                            stop=(ch == 1),
                        )
                    nc.vector.tensor_copy(kT_sb[m][:, ds(512 * j, 512)], ps[:, :512])
                for t in range(4):
                    kt = 4 * j + t
                    ps = spsum.tile([128, 1024], f32, tag="s", name=f"vp{j}_{t}")
                    for ch in range(2):
                        nc.tensor.matmul(
                            ps[:, :C],
                            lhsT=xc[:, ch, ts(t, 128)],
                            rhs=w_sb["Wv"][:, ch, :],
                            start=(ch == 0),
                            stop=(ch == 1),
                        )
                    nc.vector.tensor_copy(v_sb[:, kt, :], ps[:, :C])

            # j=0 chunks emitted before the loop; the rest are interleaved
            # into the first iterations (the loop only needs chunk j by
            # iteration 4j, and v(kt) only by iteration kt+LAG).
            emit_qproj(0)
            emit_kvproj(0)

            if dbg:
                for hp in range(4):
                    dbg_expf = nc.dram_tensor(
                        f"dbg_expf{hp}", [128, 2 * 11 * 384], bf16,
                        kind="ExternalOutput")
                    nc.sync.dma_start(out=dbg_expf[:], in_=expf_tiles[hp])

            # ---- main attention: one continuous pipeline over 128 iters ----
            oT_tiles = []
            for qc in range(2):
                oT = otp.tile([128, 2, 512], bf16, tag=f"oT{qc}", name=f"oT{qc}")
                oT_tiles.append(oT)

            ITERS = 128  # (qc 2) x (g2 4) x (kt 16)

            def it_coords(it):
                qc = it // 64
                g2 = (it // 16) % 4
                kt = it % 16
                return qc, g2, kt

            e_pend = {}    # it -> e tile
            acc_cur = [None]  # acc tile for current av block

            def emit_av(it):
                qc, g2, kt = it_coords(it)
                if kt == 0:
                    acc_cur[0] = apsum.tile(
                        [128, 512], f32, tag="acc", name=f"acc{qc}_{g2}"
                    )
                acc = acc_cur[0]
                po_av = 0 if g2 % 2 == 0 else 64
                po_rs = 64 - po_av
                e_t = e_pend.pop(it)
                for k in range(2):
                    h = 2 * g2 + k
                    nc.tensor.matmul(
                        acc[ds(po_av + 32 * k, 32), :],
                        lhsT=v_sb[:, kt, ds(32 * h, 32)],
                        rhs=e_t[:, ts(k, 512)],
                        start=(kt == 0),
                        stop=(kt == 15),
                        tile_position=(0, po_av + 32 * k),
                        skip_group_check=True,
                    )
                    nc.tensor.matmul(
                        acc[ds(po_rs + 32 * k, 32), :],
                        lhsT=ones_sb,
                        rhs=e_t[:, ts(k, 512)],
                        start=(kt == 0),
                        stop=(kt == 15),
                        tile_position=(0, po_rs + 32 * k),
                        skip_group_check=True,
                    )
                if kt == 15:
                    emit_epilogue(qc, g2, acc)

            pending_proj = []

            def emit_epilogue(qc, g2, acc):
                # recip split into two chunks so the DVE FIFO is never
                # blocked by one long instruction; av LAG cushions the rest.
                po_av = 0 if g2 % 2 == 0 else 64
                po_rs = 64 - po_av
                half_idx = g2 // 2
                oT = oT_tiles[qc]
                rep = rp.tile([128, 512], f32, tag="rep")
                for c in range(2):
                    nc.vector.reciprocal(
                        rep[ds(po_rs, 64), ds(256 * c, 256)],
                        acc[ds(po_rs, 64), ds(256 * c, 256)],
                    )
                nc.sync.dma_start(
                    out=rep[ds(po_av, 64), :], in_=rep[ds(po_rs, 64), :]
                )
                nc.vector.tensor_mul(
                    oT[ds(po_av, 64), half_idx, :],
                    acc[ds(po_av, 64), :],
                    rep[ds(po_av, 64), :],
                )
                if g2 == 3:
                    pending_proj.extend((qc, s) for s in range(4))

            def emit_out_proj_slice(qc, s):
                oT = oT_tiles[qc]
                fps = spsum.tile([128, 1024], f32, tag="s", name=f"fps{qc}_{s}")
                for ch in range(2):
                    nc.tensor.matmul(
                        fps[:, :C],
                        lhsT=oT[:, ch, ts(s, 128)],
                        rhs=w_sb["Wo"][:, ch, :],
                        start=(ch == 0),
                        stop=(ch == 1),
                    )
                stage = stp.tile([128, C], f32, tag="stage")
                nc.vector.tensor_copy(stage, fps[:, :C])
                nc.sync.dma_start(
                    out=out_d[ds(512 * qc + 128 * s, 128), :], in_=stage
                )

            for it in range(ITERS + LAG):
                if it == 4:
                    emit_qproj(1)
                    emit_kvproj(1)
                elif it == 8:
                    emit_kvproj(2)
                elif it == 12:
                    emit_kvproj(3)
                if it < ITERS:
                    qc, g2, kt = it_coords(it)
                    half_idx = g2 // 2
                    # scores -> PSUM
                    s_ps = spsum.tile([128, 1024], f32, tag="s")
                    for k in range(2):
                        h = 2 * g2 + k
                        i = h % 4
                        nc.tensor.matmul(
                            s_ps[:, ts(k, 512)],
                            lhsT=kT_sb[half_idx][ds(32 * i, 32), ts(kt, 128)],
                            rhs=qT_sb[half_idx][ds(32 * i, 32), ts(qc, 512)],
                            start=True,
                            stop=True,
                            tile_position=(32 * i, 0),
                        )
                    # exp on scalar engine
                    e_sb = ep.tile([128, 1024], bf16, tag="e")
                    e_pend[it] = e_sb
                    nc.scalar.activation(e_sb, s_ps, EXP)
                    # bias multiply on DVE
                    rdw0 = 2 * qc - (kt // 2) + 7
                    woff = 128 if kt % 2 == 0 else 0
                    ev = e_sb.rearrange("p (k jj f) -> p k jj f", k=2, jj=2)
                    fv = expf_tiles[g2].rearrange(
                        "p (h r f) -> p h r f", h=2, r=11, f=384
                    )[:, :, ds(rdw0, 2), ds(woff, 256)]
                    nc.vector.tensor_mul(ev, ev, fv)
                if it >= LAG:
                    emit_av(it - LAG)
                if pending_proj and it % 4 == 1:
                    emit_out_proj_slice(*pending_proj.pop(0))
            while pending_proj:
                emit_out_proj_slice(*pending_proj.pop(0))

    nc.compile()
    return nc


def _host_inputs(x, Wq, Wk, Wv, Wo, bias_table):
    """Build the 8 per-core input maps."""
    import ml_dtypes

    bf = ml_dtypes.bfloat16
    x = np.asarray(x, dtype=np.float32)
    T = np.asarray(bias_table, dtype=np.float32)
    xf = np.ascontiguousarray(x.reshape(B, N, C))
    idx_w = 15 + np.arange(16)[None, :] - np.arange(16)[:, None]  # [w2, w1]
    Ws = {
        "Wq": np.ascontiguousarray(np.asarray(Wq, np.float32).astype(bf)),
        "Wk": np.ascontiguousarray(np.asarray(Wk, np.float32).astype(bf)),
        "Wv": np.ascontiguousarray(np.asarray(Wv, np.float32).astype(bf)),
        "Wo": np.ascontiguousarray(np.asarray(Wo, np.float32).astype(bf)),
    }
    in_maps = []
    for c in range(8):
        b, r = c // 2, c % 2
        d1min = 4 * r
        Twin = T[:, d1min:d1min + 11]                     # [8, 11, 31, 31]
        TW = Twin[:, :, :, idx_w]                         # [8,11,31,16,16] (h,rdw,rh,w2,w1)
        TW = TW.transpose(0, 1, 3, 2, 4)                  # [h,rdw,w2,rh,w1]
        TWE = np.ascontiguousarray(
            np.exp(TW).astype(bf).reshape(4, 128, 1364))
        in_maps.append({
            "xT": np.ascontiguousarray(xf[b].T.astype(bf)),
            "xTq": np.ascontiguousarray(xf[b, QR * r:QR * (r + 1)].T.astype(bf)),
            "TWE": TWE,
            **Ws,
        })
    return in_maps


def kernel(x, Wq, Wk, Wv, Wo, bias_table, _results_hook=None):
    global _NC
    if _NC is None:
        _NC = _build_nc()
    from concourse.bass_utils import run_bass_kernel_spmd

    in_maps = _host_inputs(x, Wq, Wk, Wv, Wo, bias_table)
    res = run_bass_kernel_spmd(_NC, in_maps, core_ids=list(range(8)))
    if _results_hook is not None:
        _results_hook(res)
    out = np.zeros((B, N, C), dtype=np.float32)
    for c in range(8):
        b, r = c // 2, c % 2
        out[b, QR * r:QR * (r + 1)] = res.results[c]["out"]
    D, H, W = 8, 16, 16
    return out.reshape(B, D, H, W, C)
